# revision 14
# baseline (speedup 1.0000x reference)
"""InteractionNetwork (GNN message passing) Bass kernel for 8 Trainium2 cores.

Strategy (edge-sharded, per sharding hint):
  - Shard the 32768 edges across 8 cores (4096 each). Each core streams its
    rr/rs one-hot slices from HBM exactly once (the memory roofline),
    fp32->fp16 cast in the DMA.
  - Receiver/sender indices are recovered on-device with a fused
    tensor_tensor_reduce (one-hot dot iota) on the Vector engine; node
    features are gathered with indirect DMA; the 4-layer relation MLP runs
    feature-major on the PE; edge effects are aggregated to nodes with a
    natural-layout matmul  e_agg.T += e_chunk.T @ rr_chunk  into a pinned
    PSUM accumulator.
  - Partial e_agg is AllReduce-summed across the 8 cores; every core then
    runs the small object MLP on all 2048 nodes; host takes core 0's output.
"""

import os
import sys

import numpy as np

os.environ.setdefault("MYCRO_LOCAL_CACHE", "1")
for _p in ("/opt/trn_rl_repo",):
    if os.path.isdir(_p) and _p not in sys.path:
        sys.path.insert(0, _p)

import concourse.bacc as bacc
import concourse.bass as bass
import concourse.mybir as mybir
import concourse.tile as tile
from concourse.bass_utils import run_bass_kernel_spmd
from concourse.masks import make_identity

P = 128
F32 = mybir.dt.float32
F16 = mybir.dt.float16
I32 = mybir.dt.int32
I16 = mybir.dt.int16
AF = mybir.ActivationFunctionType
ALU = mybir.AluOpType

N_OBJ, N_REL = 2048, 32768
D_OBJ, D_REL, D_EFF = 64, 32, 64
H_REL, H_OBJ = 128, 128
D_OUT = 3
N_CORES = 8


def build(n_cores=N_CORES, e_per_core=N_REL // N_CORES, n_obj=N_OBJ,
          use_collective=True, use_indirect=True, use_ttr=True):
    EG = 512                  # edges per MLP group
    T = EG // P               # 128-edge chunks per group
    n_groups = e_per_core // EG
    NQ = 512                  # node chunk (psum bank) for wide matmuls
    n_nq = n_obj // NQ

    nc = bacc.Bacc(
        "TRN2",
        target_bir_lowering=False,
        debug=False,
        enable_asserts=False,
        num_devices=n_cores,
    )

    rr = nc.dram_tensor("rr_c", [e_per_core, n_obj], F32, kind="ExternalInput")
    rs = nc.dram_tensor("rs_c", [e_per_core, n_obj], F32, kind="ExternalInput")
    ra = nc.dram_tensor("ra_c", [e_per_core, D_REL], F32, kind="ExternalInput")
    obj = nc.dram_tensor("obj", [n_obj, D_OBJ], F32, kind="ExternalInput")
    rm_w1 = nc.dram_tensor("rm_w1", [2 * D_OBJ + D_REL, H_REL], F32, kind="ExternalInput")
    rm_b1 = nc.dram_tensor("rm_b1", [H_REL], F32, kind="ExternalInput")
    rm_w2 = nc.dram_tensor("rm_w2", [H_REL, H_REL], F32, kind="ExternalInput")
    rm_b2 = nc.dram_tensor("rm_b2", [H_REL], F32, kind="ExternalInput")
    rm_w3 = nc.dram_tensor("rm_w3", [H_REL, H_REL], F32, kind="ExternalInput")
    rm_b3 = nc.dram_tensor("rm_b3", [H_REL], F32, kind="ExternalInput")
    rm_w4 = nc.dram_tensor("rm_w4", [H_REL, D_EFF], F32, kind="ExternalInput")
    rm_b4 = nc.dram_tensor("rm_b4", [D_EFF], F32, kind="ExternalInput")
    om_w1 = nc.dram_tensor("om_w1", [D_OBJ + D_EFF, H_OBJ], F32, kind="ExternalInput")
    om_b1 = nc.dram_tensor("om_b1", [H_OBJ], F32, kind="ExternalInput")
    om_w2 = nc.dram_tensor("om_w2", [H_OBJ, D_OUT], F32, kind="ExternalInput")
    om_b2 = nc.dram_tensor("om_b2", [D_OUT], F32, kind="ExternalInput")
    pT_d = nc.dram_tensor("pT", [D_OUT, n_obj], F32, kind="ExternalOutput")

    with tile.TileContext(nc) as tc:
        with (
            tc.tile_pool(name="const", bufs=1) as const,
            tc.tile_pool(name="stream", bufs=2) as sp,
            tc.tile_pool(name="gat", bufs=4) as gp,
            tc.tile_pool(name="ec", bufs=8) as ecp,
            tc.tile_pool(name="aggp", bufs=1, space="PSUM") as aggp,
            tc.tile_pool(name="psp", bufs=4, space="PSUM") as psp,
            tc.tile_pool(name="dram", bufs=1, space="DRAM") as dp,
        ):
            # ---- constants -------------------------------------------------
            ident32 = const.tile([P, P], F32)
            make_identity(nc, ident32[:])
            ident16 = const.tile([P, P], F16)
            make_identity(nc, ident16[:])

            iota_i = const.tile([P, n_obj], I16)
            nc.gpsimd.iota(iota_i[:], pattern=[[1, n_obj]], base=0, channel_multiplier=0)
            iota16 = const.tile([P, n_obj], F16)
            nc.vector.tensor_copy(iota16[:], iota_i[:])

            w1ab = const.tile([P, H_REL], F32)
            nc.sync.dma_start(w1ab[:], rm_w1[0:P, :])
            w1c = const.tile([D_REL, H_REL], F32)
            nc.sync.dma_start(w1c[:], rm_w1[P : P + D_REL, :])
            w2 = const.tile([H_REL, H_REL], F32)
            nc.sync.dma_start(w2[:], rm_w2[:, :])
            w3 = const.tile([H_REL, H_REL], F32)
            nc.sync.dma_start(w3[:], rm_w3[:, :])
            w4 = const.tile([H_REL, D_EFF], F32)
            nc.sync.dma_start(w4[:], rm_w4[:, :])
            b1t = const.tile([H_REL, 1], F32)
            nc.sync.dma_start(b1t[:], rm_b1[:, None])
            b2t = const.tile([H_REL, 1], F32)
            nc.sync.dma_start(b2t[:], rm_b2[:, None])
            b3t = const.tile([H_REL, 1], F32)
            nc.sync.dma_start(b3t[:], rm_b3[:, None])
            b4t = const.tile([D_EFF, 1], F32)
            nc.sync.dma_start(b4t[:], rm_b4[:, None])
            ow1a = const.tile([D_OBJ, H_OBJ], F32)
            nc.sync.dma_start(ow1a[:], om_w1[0:D_OBJ, :])
            ow1b = const.tile([D_EFF, H_OBJ], F32)
            nc.sync.dma_start(ow1b[:], om_w1[D_OBJ : D_OBJ + D_EFF, :])
            ow2 = const.tile([H_OBJ, D_OUT], F32)
            nc.sync.dma_start(ow2[:], om_w2[:, :])
            ob1t = const.tile([H_OBJ, 1], F32)
            nc.sync.dma_start(ob1t[:], om_b1[:, None])
            ob2t = const.tile([D_OUT, 1], F32)
            nc.sync.dma_start(ob2t[:], om_b2[:, None])

            # obj.T in SBUF (for the node-model MLP)
            objT = const.tile([D_OBJ, n_obj], F32)
            for k in range(n_obj // P):
                ot = gp.tile([P, D_OBJ], F32, tag="objload")
                nc.sync.dma_start(ot[:], obj[k * P : (k + 1) * P, :])
                tp = psp.tile([D_OBJ, P], F32, tag="ps")
                nc.tensor.transpose(tp[:], ot[:], ident32[:])
                nc.scalar.copy(objT[:, k * P : (k + 1) * P], tp[:])

            # pinned accumulator: e_agg.T [64, n_obj] (4 PSUM banks)
            agg_ps = aggp.tile([D_EFF, n_obj], F32)

            # ---- edge phase ------------------------------------------------
            for g in range(n_groups):
                e0 = g * EG
                rrg = sp.tile([P, T, n_obj], F16, tag="rrg")
                nc.gpsimd.dma_start(
                    rrg[:], rr[e0 : e0 + EG, :].rearrange("(t p) n -> p t n", p=P)
                )
                rsg = sp.tile([P, T, n_obj], F16, tag="rsg")
                nc.gpsimd.dma_start(
                    rsg[:], rs[e0 : e0 + EG, :].rearrange("(t p) n -> p t n", p=P)
                )
                rag = sp.tile([P, T, D_REL], F32, tag="rag")
                nc.sync.dma_start(
                    rag[:], ra[e0 : e0 + EG, :].rearrange("(t p) d -> p t d", p=P)
                )

                idxf = sp.tile([P, 2 * T], F32, tag="idxf")
                idxi = sp.tile([P, 2 * T], I32, tag="idxi")
                if use_ttr:
                    for t in range(T):
                        scr = sp.tile([P, n_obj], F16, tag="scr")
                        nc.vector.tensor_tensor(
                            out=scr[:], in0=rrg[:, t, :], in1=iota16[:],
                            op=ALU.mult,
                        )
                        nc.scalar.activation(
                            scr[:], scr[:], AF.Copy,
                            accum_out=idxf[:, t : t + 1],
                        )
                        scr2 = sp.tile([P, n_obj], F16, tag="scr")
                        nc.vector.tensor_tensor(
                            out=scr2[:], in0=rsg[:, t, :], in1=iota16[:],
                            op=ALU.mult,
                        )
                        nc.scalar.activation(
                            scr2[:], scr2[:], AF.Copy,
                            accum_out=idxf[:, T + t : T + t + 1],
                        )
                else:
                    nc.gpsimd.memset(idxf[:], 0.0)
                nc.vector.tensor_copy(idxi[:], idxf[:])

                b1T = sp.tile([P, EG], F32, tag="b1T")
                raT = sp.tile([D_REL, EG], F32, tag="raT")
                for t in range(T):
                    orr_t = gp.tile([P, D_OBJ], F32, tag="gat")
                    if use_indirect:
                        nc.gpsimd.indirect_dma_start(
                            out=orr_t[:], out_offset=None, in_=obj[:, :],
                            in_offset=bass.IndirectOffsetOnAxis(
                                ap=idxi[:, t : t + 1], axis=0
                            ),
                        )
                    else:
                        nc.sync.dma_start(orr_t[:], obj[0:P, :])
                    tp = psp.tile([D_OBJ, P], F32, tag="ps")
                    nc.tensor.transpose(tp[:], orr_t[:], ident32[:])
                    nc.scalar.copy(b1T[0:D_OBJ, t * P : (t + 1) * P], tp[:])

                    ors_t = gp.tile([P, D_OBJ], F32, tag="gat")
                    if use_indirect:
                        nc.gpsimd.indirect_dma_start(
                            out=ors_t[:], out_offset=None, in_=obj[:, :],
                            in_offset=bass.IndirectOffsetOnAxis(
                                ap=idxi[:, T + t : T + t + 1], axis=0
                            ),
                        )
                    else:
                        nc.sync.dma_start(ors_t[:], obj[0:P, :])
                    tp2 = psp.tile([D_OBJ, P], F32, tag="ps")
                    nc.tensor.transpose(tp2[:], ors_t[:], ident32[:])
                    nc.scalar.copy(b1T[D_OBJ : 2 * D_OBJ, t * P : (t + 1) * P], tp2[:])

                    tp3 = psp.tile([D_REL, P], F32, tag="ps")
                    nc.tensor.transpose(tp3[:], rag[:, t, :], ident32[:])
                    nc.scalar.copy(raT[:, t * P : (t + 1) * P], tp3[:])

                # relation MLP, feature-major [features, EG]
                h1p = psp.tile([H_REL, EG], F32, tag="ps")
                nc.tensor.matmul(h1p[:], w1ab[:], b1T[:], start=True, stop=False)
                nc.tensor.matmul(h1p[:], w1c[:], raT[:], start=False, stop=True)
                h1T = sp.tile([H_REL, EG], F32, tag="hT")
                nc.scalar.activation(h1T[:], h1p[:], AF.Relu, bias=b1t[:])

                h2p = psp.tile([H_REL, EG], F32, tag="ps")
                nc.tensor.matmul(h2p[:], w2[:], h1T[:], start=True, stop=True)
                h2T = sp.tile([H_REL, EG], F32, tag="hT")
                nc.scalar.activation(h2T[:], h2p[:], AF.Relu, bias=b2t[:])

                h3p = psp.tile([H_REL, EG], F32, tag="ps")
                nc.tensor.matmul(h3p[:], w3[:], h2T[:], start=True, stop=True)
                h3T = sp.tile([H_REL, EG], F32, tag="hT")
                nc.scalar.activation(h3T[:], h3p[:], AF.Relu, bias=b3t[:])

                h4p = psp.tile([D_EFF, EG], F32, tag="ps")
                nc.tensor.matmul(h4p[:], w4[:], h3T[:], start=True, stop=True)
                eT = sp.tile([D_EFF, EG], F16, tag="eT")
                nc.scalar.activation(eT[:], h4p[:], AF.Relu, bias=b4t[:])

                # aggregate: e_agg.T += e_chunk.T @ rr_chunk
                for t in range(T):
                    ep = psp.tile([P, D_EFF], F16, tag="ps")
                    nc.tensor.transpose(
                        ep[:], eT[:, t * P : (t + 1) * P], ident16[:D_EFF, :D_EFF]
                    )
                    ec = ecp.tile([P, D_EFF], F16, tag="ec")
                    nc.scalar.copy(ec[:], ep[:])
                    first = g == 0 and t == 0
                    last = g == n_groups - 1 and t == T - 1
                    for q in range(n_obj // NQ):
                        nc.tensor.matmul(
                            agg_ps[:, q * NQ : (q + 1) * NQ],
                            ec[:],
                            rrg[:, t, q * NQ : (q + 1) * NQ],
                            start=first,
                            stop=last,
                        )

            # ---- all-reduce e_agg across cores -----------------------------
            eagg_sb = const.tile([D_EFF, n_obj], F32)
            nc.scalar.copy(eagg_sb[:], agg_ps[:])
            cc_in = dp.tile([D_EFF, n_obj], F32)
            cc_out = dp.tile([D_EFF, n_obj], F32)
            nc.sync.dma_start(cc_in[:], eagg_sb[:])
            if use_collective:
                nc.gpsimd.collective_compute(
                    "AllReduce",
                    ALU.add,
                    replica_groups=[list(range(n_cores))],
                    ins=[cc_in.opt()],
                    outs=[cc_out.opt()],
                )
            else:
                nc.sync.dma_start(cc_out[:], cc_in[:])
            eaggT = const.tile([D_EFF, n_obj], F32)
            nc.sync.dma_start(eaggT[:], cc_out[:])

            # ---- node phase (object MLP) -----------------------------------
            pTt = const.tile([D_OUT, n_obj], F32)
            for q in range(n_nq):
                sl = slice(q * NQ, (q + 1) * NQ)
                cp = psp.tile([H_OBJ, NQ], F32, tag="ps")
                nc.tensor.matmul(cp[:], ow1a[:], objT[:, sl], start=True, stop=False)
                nc.tensor.matmul(cp[:], ow1b[:], eaggT[:, sl], start=False, stop=True)
                hT = sp.tile([H_OBJ, NQ], F32, tag="hT")
                nc.scalar.activation(hT[:], cp[:], AF.Relu, bias=ob1t[:])
                pp = psp.tile([D_OUT, NQ], F32, tag="ps")
                nc.tensor.matmul(pp[:], ow2[:], hT[:], start=True, stop=True)
                nc.scalar.activation(pTt[:, sl], pp[:], AF.Identity, bias=ob2t[:])
            nc.sync.dma_start(pT_d[:, :], pTt[:])

    nc.compile()
    return nc


_CACHE = {}
TRACE = False


def _get_nc():
    if "nc" not in _CACHE:
        _CACHE["nc"] = build()
    return _CACHE["nc"]


def kernel(**inputs):
    nc = _get_nc()
    f = lambda k: np.ascontiguousarray(np.asarray(inputs[k], dtype=np.float32))
    obj = f("obj")
    shared = {
        "obj": obj,
        "rm_w1": f("rm_w1"), "rm_b1": f("rm_b1"),
        "rm_w2": f("rm_w2"), "rm_b2": f("rm_b2"),
        "rm_w3": f("rm_w3"), "rm_b3": f("rm_b3"),
        "rm_w4": f("rm_w4"), "rm_b4": f("rm_b4"),
        "om_w1": f("om_w1"), "om_b1": f("om_b1"),
        "om_w2": f("om_w2"), "om_b2": f("om_b2"),
    }
    rr = f("rr")
    rs = f("rs")
    ra = f("ra")
    epc = N_REL // N_CORES
    in_maps = []
    for c in range(N_CORES):
        sl = slice(c * epc, (c + 1) * epc)
        m = dict(shared)
        m["rr_c"] = np.ascontiguousarray(rr[sl])
        m["rs_c"] = np.ascontiguousarray(rs[sl])
        m["ra_c"] = np.ascontiguousarray(ra[sl])
        in_maps.append(m)
    res = run_bass_kernel_spmd(
        nc, in_maps, core_ids=list(range(N_CORES)), trace=TRACE
    )
    _CACHE["last_results"] = res
    return np.ascontiguousarray(res.results[0]["pT"].T)
